# revision 1
# baseline (speedup 1.0000x reference)
"""Trainium2 Bass kernel: GNN message passing  out = relu((adj @ x) @ W.T + b).

Sharding: 1D row partition of adj across 8 NeuronCores (1024 rows each).
Each core computes aggT = x.T @ adjT_c (= (adj_c @ x).T) with x-tiles as the
stationary operand and the pre-transposed adj shard streaming as the moving
operand, accumulating f32 in PSUM over the 8192-deep contraction.  The second
linear runs as outT = (W.T-tiles) @ aggT so the bias lands on the partition
dim, letting the scalar engine fuse bias+ReLU while reading PSUM.  Host-side
numpy does the adj transpose + bf16 casts and re-assembles the full output.
"""

import numpy as np
import ml_dtypes

import concourse.mybir as mybir
from concourse import bacc
from concourse.tile import TileContext
from concourse.bass_utils import run_bass_kernel_spmd

P = 128
N_NODES = 8192
DIM = 512
NCORES = 8
M = N_NODES // NCORES          # 1024 output rows per core
KT = N_NODES // P              # 64 contraction tiles
NT = DIM // P                  # 4 tiles of the hidden dim (MM1 output part.)
JT = DIM // P                  # 4 tiles of the output-feature dim
FREE = 512                     # moving free dim / PSUM bank width (f32)
MCH = M // FREE                # 2 moving chunks per adj tile row block
BF16 = mybir.dt.bfloat16
F32 = mybir.dt.float32

_NC = None


def _build_nc():
    nc = bacc.Bacc("TRN2", debug=False)
    x_d = nc.dram_tensor("x", [N_NODES, DIM], BF16, kind="ExternalInput").ap()
    adjt_d = nc.dram_tensor("adjt", [N_NODES, M], BF16, kind="ExternalInput").ap()
    wt_d = nc.dram_tensor("wt", [DIM, DIM], BF16, kind="ExternalInput").ap()
    b_d = nc.dram_tensor("b", [P, JT], F32, kind="ExternalInput").ap()
    out_d = nc.dram_tensor("outt", [DIM, M], F32, kind="ExternalOutput").ap()

    with TileContext(nc) as tc:
        with (
            tc.tile_pool(name="xsb", bufs=1) as xpool,
            tc.tile_pool(name="wsb", bufs=1) as wpool,
            tc.tile_pool(name="adjh", bufs=9) as adjhpool,
            tc.tile_pool(name="adj", bufs=7) as adjpool,
            tc.tile_pool(name="agg", bufs=1) as aggpool,
            tc.tile_pool(name="osb", bufs=4) as opool,
            tc.tile_pool(name="ps", bufs=8, space="PSUM") as pspool,
        ):
            # Resident stationary operands: x (64 KB/part) and W.T (4 KB/part).
            # x tile loads are interleaved into the k loop below so the 8 MiB
            # x preload doesn't starve the adj stream.
            x_sb = xpool.tile([P, KT * DIM], BF16)
            wt_sb = wpool.tile([P, NT * DIM], BF16)
            for n in range(NT):
                nc.sync.dma_start(
                    wt_sb[:, n * DIM : (n + 1) * DIM], wt_d[n * P : (n + 1) * P, :]
                )
            b_sb = wpool.tile([P, JT], F32)
            nc.sync.dma_start(b_sb[:], b_d[:])

            # MM1: aggT[n*128+a, mc*512+m] accumulated in 8 PSUM banks.
            agg_ps = [
                [
                    pspool.tile([P, FREE], F32, tag="ps", name=f"aggps_{n}_{mc}")
                    for mc in range(MCH)
                ]
                for n in range(NT)
            ]
            # Ramp phase (k < RAMP): one k-tile per DMA, adj split in halves —
            # small descriptors land fast across many queues so the PE starts
            # within ~3us.  Steady phase: two k-tiles per descriptor to halve
            # the sync sequencer issue load (~600ns per dma_start) so prefetch
            # depth builds instead of issue-rate-limiting the stream.
            RAMP = 8

            def mm1_ktile(k, adj_tile, off):
                for n in range(NT):
                    for mc in range(MCH):
                        nc.tensor.matmul(
                            agg_ps[n][mc][:],
                            x_sb[:, k * DIM + n * P : k * DIM + (n + 1) * P],
                            adj_tile[:, off + mc * FREE : off + (mc + 1) * FREE],
                            start=(k == 0),
                            stop=(k == KT - 1),
                        )

            for k in range(RAMP):
                nc.sync.dma_start(
                    x_sb[:, k * DIM : (k + 1) * DIM], x_d[k * P : (k + 1) * P, :]
                )
                adj_sb = adjhpool.tile([P, M], BF16, tag="adjh", name=f"adjh_{k}")
                nc.sync.dma_start(adj_sb[:, :FREE], adjt_d[k * P : (k + 1) * P, :FREE])
                nc.sync.dma_start(adj_sb[:, FREE:], adjt_d[k * P : (k + 1) * P, FREE:])
                mm1_ktile(k, adj_sb, 0)

            for k2 in range(RAMP // 2, KT // 2):
                nc.sync.dma_start(
                    x_sb[:, k2 * 2 * DIM : (k2 + 1) * 2 * DIM].rearrange(
                        "p (two n) -> p two n", two=2
                    ),
                    x_d[k2 * 2 * P : (k2 + 1) * 2 * P, :].rearrange(
                        "(two p) n -> p two n", p=P
                    ),
                )
                adj_sb = adjpool.tile([P, 2 * M], BF16, tag="adj", name=f"adj_{k2}")
                nc.sync.dma_start(
                    adj_sb[:].rearrange("p (two m) -> p two m", two=2),
                    adjt_d[k2 * 2 * P : (k2 + 1) * 2 * P, :].rearrange(
                        "(two p) m -> p two m", p=P
                    ),
                )
                for two in range(2):
                    mm1_ktile(2 * k2 + two, adj_sb, two * M)

            # PSUM -> SBUF (cast to bf16) so MM2 can stream aggT as moving op.
            agg_sb = aggpool.tile([P, NT * M], BF16)
            for n in range(NT):
                for mc in range(MCH):
                    nc.vector.tensor_copy(
                        agg_sb[:, n * M + mc * FREE : n * M + (mc + 1) * FREE],
                        agg_ps[n][mc][:],
                    )

            # MM2 + bias/ReLU epilogue (alternating ACT and DVE so the final
            # chain isn't serialized on one engine), stores paired per two
            # j-tiles into one 3D descriptor to halve tail issue cost.
            for mc in range(MCH):
                for j2 in range(JT // 2):
                    pair_sb = opool.tile(
                        [P, 2 * FREE], F32, tag="osb", name=f"o_{mc}_{j2}"
                    )
                    for jj in range(2):
                        j = 2 * j2 + jj
                        out_ps = pspool.tile(
                            [P, FREE], F32, tag="ps", name=f"ops_{mc}_{j}"
                        )
                        for n in range(NT):
                            nc.tensor.matmul(
                                out_ps[:],
                                wt_sb[:, n * DIM + j * P : n * DIM + (j + 1) * P],
                                agg_sb[:, n * M + mc * FREE : n * M + (mc + 1) * FREE],
                                start=(n == 0),
                                stop=(n == NT - 1),
                            )
                        if jj == 0:
                            nc.scalar.activation(
                                pair_sb[:, :FREE],
                                out_ps[:],
                                mybir.ActivationFunctionType.Relu,
                                bias=b_sb[:, j : j + 1],
                            )
                        else:
                            nc.vector.tensor_scalar(
                                pair_sb[:, FREE:],
                                out_ps[:],
                                b_sb[:, j : j + 1],
                                0.0,
                                mybir.AluOpType.add,
                                mybir.AluOpType.max,
                            )
                    nc.sync.dma_start(
                        out_d[
                            j2 * 2 * P : (j2 + 1) * 2 * P,
                            mc * FREE : (mc + 1) * FREE,
                        ].rearrange("(two p) m -> p two m", p=P),
                        pair_sb[:].rearrange("p (two m) -> p two m", two=2),
                    )
    nc.finalize()
    return nc


def _get_nc():
    global _NC
    if _NC is None:
        _NC = _build_nc()
    return _NC


def _prepare(inputs):
    bf = ml_dtypes.bfloat16
    x = np.asarray(inputs["x"], dtype=np.float32)
    adj = np.asarray(inputs["adj"], dtype=np.float32)
    W = np.asarray(inputs["W"], dtype=np.float32)
    b = np.asarray(inputs["b"], dtype=np.float32)

    x_bf = np.ascontiguousarray(x.astype(bf))
    wt_bf = np.ascontiguousarray(W.T.astype(bf))
    adjt_bf = adj.astype(bf).T  # [K, rows] view
    b_tiled = np.ascontiguousarray(b.reshape(JT, P).T)  # [128, 4]

    in_maps = []
    for c in range(NCORES):
        in_maps.append(
            {
                "x": x_bf,
                "adjt": np.ascontiguousarray(adjt_bf[:, c * M : (c + 1) * M]),
                "wt": wt_bf,
                "b": b_tiled,
            }
        )
    return in_maps


def _run(in_maps, **kwargs):
    return run_bass_kernel_spmd(
        _get_nc(), in_maps, core_ids=list(range(NCORES)), **kwargs
    )


def _assemble(results):
    out = np.empty((N_NODES, DIM), dtype=np.float32)
    for c in range(NCORES):
        out[c * M : (c + 1) * M, :] = results[c]["outt"].T
    return out


def kernel(**inputs):
    res = _run(_prepare(inputs))
    return _assemble(res.results)



# revision 2
# speedup vs baseline: 1.6684x; 1.6684x over previous
"""Trainium2 Bass kernel: GNN message passing  out = relu((adj @ x) @ W.T + b).

Reassociated as  out = relu(adj @ y + b)  with y = x @ W.T folded into host
prep (0.8% of the FLOPs), so the device runs a single big matmul.  That
matmul runs in fp8e4 DoubleRow mode (2 weights per PE cell, 256-deep
contraction per instruction, ~1.44x bf16 throughput).  fp8 error is held at
~1.8e-2 by two exact algebraic corrections folded into the bias:
  * adj is mean-centered (B = adj - 0.5), halving its quantization noise;
    the rank-1 term 0.5 * colsum(y) is exact.
  * using colsum(y_true) rather than colsum(y_fp8) also cancels the coherent
    (mean) component of y's quantization error, halving the y-side noise.
Sharding: 1D row partition of adj across 8 NeuronCores (1024 rows each);
each core computes outT = y.T @ adjT_c with y-tiles stationary and the
pre-transposed centered adj shard streaming, accumulating f32 in PSUM over
the 8192-deep contraction, then fuses bias+ReLU on PSUM eviction.
"""

import numpy as np
import ml_dtypes

import concourse.mybir as mybir
from concourse import bacc
from concourse.tile import TileContext
from concourse.bass_utils import run_bass_kernel_spmd

P = 128
N_NODES = 8192
DIM = 512
NCORES = 8
M = N_NODES // NCORES          # 1024 output rows per core
KT = N_NODES // P              # 64 contraction tiles of 128
DT = KT // 2                   # 32 DoubleRow tiles (256-deep each)
NT = DIM // P                  # 4 tiles of the feature dim (output part.)
FREE = 512                     # moving free dim / PSUM bank width (f32)
MCH = M // FREE                # 2 moving chunks per adj tile row block
FP8 = mybir.dt.float8e4
F32 = mybir.dt.float32
DR = mybir.MatmulPerfMode.DoubleRow

_NC = None


def _build_nc():
    nc = bacc.Bacc("TRN2", debug=False)
    y_d = nc.dram_tensor("y", [N_NODES, DIM], FP8, kind="ExternalInput").ap()
    adjt_d = nc.dram_tensor("adjt", [N_NODES, M], FP8, kind="ExternalInput").ap()
    cb_d = nc.dram_tensor("cb", [P, NT], F32, kind="ExternalInput").ap()
    out_d = nc.dram_tensor("outt", [DIM, M], F32, kind="ExternalOutput").ap()

    with TileContext(nc) as tc:
        with (
            tc.tile_pool(name="ysb", bufs=1) as ypool,
            tc.tile_pool(name="adjh", bufs=5) as adjhpool,
            tc.tile_pool(name="adj", bufs=6) as adjpool,
            tc.tile_pool(name="osb", bufs=4) as opool,
            tc.tile_pool(name="ps", bufs=8, space="PSUM") as pspool,
        ):
            # Stationary y (4 MiB, 32 KB/partition) resident in SBUF as
            # [128, ktile, 512]; DoubleRow slices [*, 2t:2t+2, n*128:+128].
            y_sb = ypool.tile([P, KT, DIM], FP8)
            cb_sb = ypool.tile([P, NT], F32)
            nc.sync.dma_start(cb_sb[:], cb_d[:])

            agg_ps = [
                [
                    pspool.tile([P, FREE], F32, tag="ps", name=f"ps_{n}_{mc}")
                    for mc in range(MCH)
                ]
                for n in range(NT)
            ]

            def load_y(k0, k1):
                nc.sync.dma_start(
                    y_sb[:, k0:k1, :],
                    y_d[k0 * P : k1 * P, :].rearrange("(g p) d -> p g d", p=P),
                )

            def load_adj(t, pool, tag, halves):
                adj_sb = pool.tile([P, 2, M], FP8, tag=tag, name=f"adj_{t}")
                src = adjt_d[2 * t * P : (2 * t + 2) * P, :]
                if halves:
                    nc.sync.dma_start(
                        adj_sb[:, :, :FREE],
                        src[:, :FREE].rearrange("(two p) m -> p two m", p=P),
                    )
                    nc.sync.dma_start(
                        adj_sb[:, :, FREE:],
                        src[:, FREE:].rearrange("(two p) m -> p two m", p=P),
                    )
                else:
                    nc.sync.dma_start(
                        adj_sb[:], src.rearrange("(two p) m -> p two m", p=P)
                    )
                return adj_sb

            def mm_tile(t, adj_sb):
                for n in range(NT):
                    for mc in range(MCH):
                        nc.tensor.matmul(
                            agg_ps[n][mc][:],
                            y_sb[:, 2 * t : 2 * t + 2, n * P : (n + 1) * P],
                            adj_sb[:, :, mc * FREE : (mc + 1) * FREE],
                            start=(t == 0),
                            stop=(t == DT - 1),
                            perf_mode=DR,
                        )

            # Ramp: small y/adj descriptors across many queues so the PE
            # starts fast; steady: one 256 KB descriptor per double-tile,
            # y front-loaded in 8-ktile chunks between adj tiles.
            RAMP = 4
            load_y(0, 2)
            for t in range(RAMP):
                load_y(2 * t + 2, 2 * t + 4)
                adj_sb = load_adj(t, adjhpool, "adjh", halves=True)
                mm_tile(t, adj_sb)

            next_k = 2 * RAMP + 2
            for t in range(RAMP, DT):
                if next_k < KT:
                    load_y(next_k, min(next_k + 8, KT))
                    next_k += 8
                adj_sb = load_adj(t, adjpool, "adj", halves=False)
                mm_tile(t, adj_sb)

            # Epilogue: bias+ReLU on PSUM eviction, alternating ACT and DVE
            # so the tail isn't serialized on one engine; paired stores.
            for mc in range(MCH):
                for a in range(NT // 2):
                    pair_sb = opool.tile(
                        [P, 2 * FREE], F32, tag="osb", name=f"o_{mc}_{a}"
                    )
                    for jj in range(2):
                        n = 2 * a + jj
                        if jj == 0:
                            nc.scalar.activation(
                                pair_sb[:, :FREE],
                                agg_ps[n][mc][:],
                                mybir.ActivationFunctionType.Relu,
                                bias=cb_sb[:, n : n + 1],
                            )
                        else:
                            nc.vector.tensor_scalar(
                                pair_sb[:, FREE:],
                                agg_ps[n][mc][:],
                                cb_sb[:, n : n + 1],
                                0.0,
                                mybir.AluOpType.add,
                                mybir.AluOpType.max,
                            )
                    nc.sync.dma_start(
                        out_d[
                            a * 2 * P : (a + 1) * 2 * P,
                            mc * FREE : (mc + 1) * FREE,
                        ].rearrange("(two p) m -> p two m", p=P),
                        pair_sb[:].rearrange("p (two m) -> p two m", two=2),
                    )
    nc.finalize()
    return nc


def _get_nc():
    global _NC
    if _NC is None:
        _NC = _build_nc()
    return _NC


def _prepare(inputs):
    e4 = ml_dtypes.float8_e4m3
    x = np.asarray(inputs["x"], dtype=np.float32)
    adj = np.asarray(inputs["adj"], dtype=np.float32)
    W = np.asarray(inputs["W"], dtype=np.float32)
    b = np.asarray(inputs["b"], dtype=np.float64)

    y = x @ W.T.astype(np.float32)
    y8 = y.astype(e4)
    # bias fold: nn bias + exact centering/rank-1 correction term
    c = (b + 0.5 * y.astype(np.float64).sum(axis=0)).astype(np.float32)
    cb_tiled = np.ascontiguousarray(c.reshape(NT, P).T)  # [128, 4]

    adjt8 = (adj - np.float32(0.5)).astype(e4).T  # [K, rows] view

    in_maps = []
    for ci in range(NCORES):
        in_maps.append(
            {
                "y": y8,
                "adjt": np.ascontiguousarray(adjt8[:, ci * M : (ci + 1) * M]),
                "cb": cb_tiled,
            }
        )
    return in_maps


def _run(in_maps, **kwargs):
    return run_bass_kernel_spmd(
        _get_nc(), in_maps, core_ids=list(range(NCORES)), **kwargs
    )


def _assemble(results):
    out = np.empty((N_NODES, DIM), dtype=np.float32)
    for ci in range(NCORES):
        out[ci * M : (ci + 1) * M, :] = results[ci]["outt"].T
    return out


def kernel(**inputs):
    res = _run(_prepare(inputs))
    return _assemble(res.results)


# revision 4
# speedup vs baseline: 1.7543x; 1.0515x over previous
"""Trainium2 Bass kernel: GNN message passing  out = relu((adj @ x) @ W.T + b).

Reassociated as  out = relu(adj @ y + b)  with y = x @ W.T folded into host
prep (0.8% of the FLOPs), so the device runs a single big matmul.  That
matmul runs in fp8e4 DoubleRow mode (2 weights per PE cell, 256-deep
contraction per instruction, ~216 ns per [256x128]x[256x512] MM).  fp8
error is held at ~1.8e-2 by two exact algebraic corrections folded into
the bias:
  * adj is mean-centered (B = adj - 0.5), halving its quantization noise;
    the rank-1 term 0.5 * colsum(y) is exact.
  * using colsum(y_true) rather than colsum(y_fp8) also cancels the
    coherent (mean) component of y's quantization error, halving it.
Sharding: 1D row partition of adj across 8 NeuronCores (1024 rows each);
each core computes outT = y.T @ adjT_c with y-tiles stationary and the
centered adj shard streaming, accumulating f32 in all 8 PSUM banks over
the 8192-deep contraction, then fuses bias+ReLU on PSUM eviction.

Perf notes (from NTFF profile): DMA descriptor issue is ~650 ns and
strictly serial per engine, so input streams are pre-interleaved on the
host into flat per-partition-contiguous layouts and issued from four
different engines (y: vector, adj: sync/scalar alternating, bias:
gpsimd, stores: spread) to keep the PE fed; a few memset-fed dummy
matmuls warm the PE HAM clock gate during the DMA ramp.
"""

import numpy as np
import ml_dtypes

import concourse.mybir as mybir
from concourse import bacc
from concourse.tile import TileContext
from concourse.bass_utils import run_bass_kernel_spmd

P = 128
N_NODES = 8192
DIM = 512
NCORES = 8
M = N_NODES // NCORES          # 1024 output rows per core
KT = N_NODES // P              # 64 contraction tiles of 128
DT = KT // 2                   # 32 DoubleRow tiles (256-deep each)
NT = DIM // P                  # 4 tiles of the feature dim (output part.)
FREE = 512                     # moving free dim / PSUM bank width (f32)
MCH = M // FREE                # 2 moving chunks per adj tile row block
WARM = 6                       # HAM warm-up matmuls on zero scratch
FP8 = mybir.dt.float8e4
F32 = mybir.dt.float32
DR = mybir.MatmulPerfMode.DoubleRow

_NC = None


def _build_nc():
    nc = bacc.Bacc("TRN2", debug=False)
    # yp[p, k*DIM+d] = y[k*128+p, d]; adjp[t, p, i*M+m] = B.T[(2t+i)*128+p, m]
    yp_d = nc.dram_tensor("yp", [P, KT * DIM], FP8, kind="ExternalInput").ap()
    adjp_d = nc.dram_tensor("adjp", [DT, P, 2 * M], FP8, kind="ExternalInput").ap()
    cb_d = nc.dram_tensor("cb", [P, NT], F32, kind="ExternalInput").ap()
    # out5[mc, a, p, jj, m] = outT[a*256+jj*128+p, mc*512+m]
    out_d = nc.dram_tensor(
        "out5", [MCH, NT // 2, P, 2, FREE], F32, kind="ExternalOutput"
    ).ap()

    with TileContext(nc) as tc:
        with (
            tc.tile_pool(name="ysb", bufs=1) as ypool,
            tc.tile_pool(name="adj", bufs=10) as adjpool,
            tc.tile_pool(name="osb", bufs=4) as opool,
            tc.tile_pool(name="ps", bufs=8, space="PSUM") as pspool,
        ):
            # Stationary y (4 MiB, 32 KB/partition) resident in SBUF as
            # [128, ktile, 512]; DoubleRow slices [*, 2t:2t+2, n*128:+128].
            y_sb = ypool.tile([P, KT, DIM], FP8)
            cb_sb = ypool.tile([P, NT], F32)
            scr_sb = ypool.tile([P, 2, 128 + FREE], FP8)

            nc.vector.memset(scr_sb[:], 0)
            nc.gpsimd.dma_start(cb_sb[:], cb_d[:])

            agg_ps = [
                [
                    pspool.tile([P, FREE], F32, tag="ps", name=f"ps_{n}_{mc}")
                    for mc in range(MCH)
                ]
                for n in range(NT)
            ]

            # Dummy matmuls on zeroed scratch keep the PE busy through the
            # DMA ramp so the HAM clock gate reaches 8/8 before real work.
            for w in range(WARM):
                nc.tensor.matmul(
                    agg_ps[0][0][:],
                    scr_sb[:, :, :P],
                    scr_sb[:, :, P:],
                    start=True,
                    stop=True,
                    perf_mode=DR,
                )

            def load_y(k0, k1):
                nc.gpsimd.dma_start(
                    y_sb[:, k0:k1, :], yp_d[:, k0 * DIM : k1 * DIM]
                )

            def load_adj(t):
                adj_sb = adjpool.tile([P, 2, M], FP8, tag="adj", name=f"adj_{t}")
                eng = nc.sync if t % 2 == 0 else nc.scalar
                eng.dma_start(
                    adj_sb[:], adjp_d[t].rearrange("p (two m) -> p two m", two=2)
                )
                return adj_sb

            def mm_tile(t, adj_sb):
                for n in range(NT):
                    for mc in range(MCH):
                        nc.tensor.matmul(
                            agg_ps[n][mc][:],
                            y_sb[:, 2 * t : 2 * t + 2, n * P : (n + 1) * P],
                            adj_sb[:, :, mc * FREE : (mc + 1) * FREE],
                            start=(t == 0),
                            stop=(t == DT - 1),
                            perf_mode=DR,
                        )

            # y front-loaded in chunks between adj tiles; adj double-
            # buffered ~10 deep.  All issues on distinct engines so no
            # serial descriptor-write chain gates the PE.
            load_y(0, 2)
            next_k = 2
            for t in range(DT):
                if next_k < KT:
                    k1 = min(next_k + 8, KT)
                    load_y(next_k, k1)
                    next_k = k1
                adj_sb = load_adj(t)
                mm_tile(t, adj_sb)

            # Epilogue: bias+ReLU on PSUM eviction, alternating ACT and DVE
            # so the tail isn't serialized on one engine; paired stores on
            # four different issue engines.
            store_eng = [nc.gpsimd, nc.sync, nc.scalar, nc.gpsimd]
            for mc in range(MCH):
                for a in range(NT // 2):
                    pair_sb = opool.tile(
                        [P, 2 * FREE], F32, tag="osb", name=f"o_{mc}_{a}"
                    )
                    for jj in range(2):
                        n = 2 * a + jj
                        if jj == 0:
                            nc.scalar.activation(
                                pair_sb[:, :FREE],
                                agg_ps[n][mc][:],
                                mybir.ActivationFunctionType.Relu,
                                bias=cb_sb[:, n : n + 1],
                            )
                        else:
                            nc.vector.tensor_scalar(
                                pair_sb[:, FREE:],
                                agg_ps[n][mc][:],
                                cb_sb[:, n : n + 1],
                                0.0,
                                mybir.AluOpType.add,
                                mybir.AluOpType.max,
                            )
                    store_eng[mc * (NT // 2) + a].dma_start(
                        out_d[mc, a],
                        pair_sb[:].rearrange("p (two m) -> p two m", two=2),
                    )
    nc.finalize()
    return nc


def _get_nc():
    global _NC
    if _NC is None:
        _NC = _build_nc()
    return _NC


def _prepare(inputs):
    e4 = ml_dtypes.float8_e4m3
    x = np.asarray(inputs["x"], dtype=np.float32)
    adj = np.asarray(inputs["adj"], dtype=np.float32)
    W = np.asarray(inputs["W"], dtype=np.float32)
    b = np.asarray(inputs["b"], dtype=np.float64)

    y = x @ W.T.astype(np.float32)
    y8 = y.astype(e4)
    # bias fold: nn bias + exact centering/rank-1 correction term
    c = (b + 0.5 * y.astype(np.float64).sum(axis=0)).astype(np.float32)
    cb_tiled = np.ascontiguousarray(c.reshape(NT, P).T)  # [128, 4]

    # y pre-tiled so every y DMA is flat: yp[p, k*DIM+d] = y8[k*128+p, d]
    yp = np.ascontiguousarray(
        y8.reshape(KT, P, DIM).transpose(1, 0, 2).reshape(P, KT * DIM)
    )

    B8T = (adj - np.float32(0.5)).astype(e4).T  # [K, rows] view

    in_maps = []
    for ci in range(NCORES):
        # adjp[t, p, i*M+m] = B8T[(2t+i)*128+p, ci*M+m]
        shard = np.ascontiguousarray(B8T[:, ci * M : (ci + 1) * M])
        adjp = np.ascontiguousarray(
            shard.reshape(DT, 2, P, M).transpose(0, 2, 1, 3).reshape(DT, P, 2 * M)
        )
        in_maps.append({"yp": yp, "adjp": adjp, "cb": cb_tiled})
    return in_maps


def _run(in_maps, **kwargs):
    return run_bass_kernel_spmd(
        _get_nc(), in_maps, core_ids=list(range(NCORES)), **kwargs
    )


def _assemble(results):
    out = np.empty((N_NODES, DIM), dtype=np.float32)
    for ci in range(NCORES):
        o5 = results[ci]["out5"]  # [MCH, NT//2, P, 2, FREE]
        outT = o5.transpose(1, 3, 2, 0, 4).reshape(DIM, M)
        out[ci * M : (ci + 1) * M, :] = outT.T
    return out


def kernel(**inputs):
    res = _run(_prepare(inputs))
    return _assemble(res.results)


# revision 9
# speedup vs baseline: 1.8230x; 1.0392x over previous
"""Trainium2 Bass kernel: GNN message passing  out = relu((adj @ x) @ W.T + b).

Reassociated as  out = relu(adj @ y + b)  with y = x @ W.T folded into host
prep (0.8% of the FLOPs), so the device runs a single big matmul.  That
matmul runs in fp8e4 DoubleRow mode (2 weights per PE cell, 256-deep
contraction per instruction, ~216 ns per [256x128]x[256x512] MM).  fp8
error is held at ~1.8e-2 by two exact algebraic corrections folded into
the bias:
  * adj is mean-centered (B = adj - 0.5), halving its quantization noise;
    the rank-1 term 0.5 * colsum(y) is exact.
  * using colsum(y_true) rather than colsum(y_fp8) also cancels the
    coherent (mean) component of y's quantization error, halving it.
Sharding: 1D row partition of adj across 8 NeuronCores (1024 rows each);
each core computes outT = y.T @ adjT_c with y-tiles stationary and the
centered adj shard streaming, accumulating f32 in all 8 PSUM banks over
the 8192-deep contraction, then fuses bias+ReLU on PSUM eviction.

Perf notes (from NTFF profile): DMA descriptor issue is ~650 ns serial
per engine and transfers drain FIFO through a shared ~358 GB/s queue
ring that only starts ~8.5 us in, so (a) input streams are
pre-interleaved on the host into flat per-partition-contiguous layouts,
(b) the first real tile's bytes are issued before anything else, from
three engines in parallel (adj halves: sync+scalar, y: gpsimd), and
(c) a handful of dummy matmuls on scratch keep the PE busy from the end
of its preamble so the HAM clock gate is at 8/8 when real data lands.
Output is stored bf16 to halve the post-matmul drain.
"""

import numpy as np
import ml_dtypes

import concourse.mybir as mybir
from concourse import bacc
from concourse.tile import TileContext
from concourse.bass_utils import run_bass_kernel_spmd

P = 128
N_NODES = 8192
DIM = 512
NCORES = 8
M = N_NODES // NCORES          # 1024 output rows per core
KT = N_NODES // P              # 64 contraction tiles of 128
DT = KT // 2                   # 32 DoubleRow tiles (256-deep each)
NT = DIM // P                  # 4 tiles of the feature dim (output part.)
FREE = 512                     # moving free dim / PSUM bank width (f32)
MCH = M // FREE                # 2 moving chunks per adj tile row block
WARM = 5                       # HAM warm-up matmuls on scratch
FP8 = mybir.dt.float8e4
F32 = mybir.dt.float32
BF16 = mybir.dt.bfloat16
DR = mybir.MatmulPerfMode.DoubleRow

_NC = None


def _build_nc():
    nc = bacc.Bacc("TRN2", debug=False)
    # yp[p, k*DIM+d] = y[k*128+p, d]
    yp_d = nc.dram_tensor("yp", [P, KT * DIM], FP8, kind="ExternalInput").ap()
    # adjp[t, p, mc, i, mm] = B.T[(2t+i)*128+p, mc*512+mm]
    adjp_d = nc.dram_tensor(
        "adjp", [DT, P, MCH, 2, FREE], FP8, kind="ExternalInput"
    ).ap()
    cb_d = nc.dram_tensor("cb", [P, NT], F32, kind="ExternalInput").ap()
    # out5[mc, a, p, jj, m] = outT[a*256+jj*128+p, mc*512+m]
    out_d = nc.dram_tensor(
        "out5", [MCH, NT // 2, P, 2, FREE], BF16, kind="ExternalOutput"
    ).ap()

    with TileContext(nc) as tc:
        with (
            tc.tile_pool(name="ysb", bufs=1) as ypool,
            tc.tile_pool(name="adj", bufs=10) as adjpool,
            tc.tile_pool(name="osb", bufs=4) as opool,
            tc.tile_pool(name="ps", bufs=8, space="PSUM") as pspool,
        ):
            # Stationary y (4 MiB, 32 KB/partition) resident in SBUF as
            # [128, ktile, 512]; DoubleRow slices [*, 2t:2t+2, n*128:+128].
            y_sb = ypool.tile([P, KT, DIM], FP8)
            cb_sb = ypool.tile([P, NT], F32)
            scr_sb = ypool.tile([P, 2, P + FREE], FP8)

            nc.vector.memset(scr_sb[:], 0)

            agg_ps = [
                [
                    pspool.tile([P, FREE], F32, tag="ps", name=f"ps_{n}_{mc}")
                    for mc in range(MCH)
                ]
                for n in range(NT)
            ]

            # Dummy matmuls on (uninitialized) scratch keep the PE busy from
            # the end of its preamble so the HAM clock gate reaches 8/8
            # before real data lands; results are cleared by start=True.
            for w in range(WARM):
                nc.tensor.matmul(
                    agg_ps[0][0][:],
                    scr_sb[:, :, :P],
                    scr_sb[:, :, P:],
                    start=True,
                    stop=True,
                    perf_mode=DR,
                )

            def load_y(k0, k1):
                nc.gpsimd.dma_start(
                    y_sb[:, k0:k1, :], yp_d[:, k0 * DIM : k1 * DIM]
                )

            def load_adj(t):
                adj_sb = adjpool.tile(
                    [P, MCH, 2, FREE], FP8, tag="adj", name=f"adj_{t}"
                )
                nc.sync.dma_start(adj_sb[:, 0], adjp_d[t][:, 0])
                nc.scalar.dma_start(adj_sb[:, 1], adjp_d[t][:, 1])
                return adj_sb

            def mm_tile(t, adj_sb):
                for mc in range(MCH):
                    for n in range(NT):
                        nc.tensor.matmul(
                            agg_ps[n][mc][:],
                            y_sb[:, 2 * t : 2 * t + 2, n * P : (n + 1) * P],
                            adj_sb[:, mc],
                            start=(t == 0),
                            stop=(t == DT - 1),
                            perf_mode=DR,
                        )

            # Issue priority: first tile's bytes before everything else
            # (the DMA ring drains roughly FIFO); y front-loaded in chunks
            # between adj tiles; bias (needed only by the epilogue) last.
            load_y(0, 2)
            y_chunks = [(2, 6), (6, 14), (14, 22), (22, 30), (30, 38),
                        (38, 46), (46, 54), (54, 64)]
            adj0 = load_adj(0)
            load_y(*y_chunks[0])
            nc.gpsimd.dma_start(cb_sb[:], cb_d[:])
            mm_tile(0, adj0)
            for t in range(1, DT):
                ci = t
                if ci < len(y_chunks):
                    load_y(*y_chunks[ci])
                adj_sb = load_adj(t)
                mm_tile(t, adj_sb)

            # Epilogue: bias+ReLU on PSUM eviction spread over three
            # engines; paired bf16 stores on parallel issue engines.
            store_eng = [nc.gpsimd, nc.sync, nc.scalar, nc.gpsimd]
            evict_eng = [nc.vector, nc.vector]
            for mc in range(MCH):
                for a in range(NT // 2):
                    pair_sb = opool.tile(
                        [P, 2 * FREE], BF16, tag="osb", name=f"o_{mc}_{a}"
                    )
                    for jj in range(2):
                        n = 2 * a + jj
                        if jj == 0:
                            nc.scalar.activation(
                                pair_sb[:, :FREE],
                                agg_ps[n][mc][:],
                                mybir.ActivationFunctionType.Relu,
                                bias=cb_sb[:, n : n + 1],
                            )
                        else:
                            evict_eng[mc].tensor_scalar(
                                pair_sb[:, FREE:],
                                agg_ps[n][mc][:],
                                cb_sb[:, n : n + 1],
                                0.0,
                                mybir.AluOpType.add,
                                mybir.AluOpType.max,
                            )
                    store_eng[mc * (NT // 2) + a].dma_start(
                        out_d[mc, a],
                        pair_sb[:].rearrange("p (two m) -> p two m", two=2),
                    )
    nc.finalize()
    return nc


def _get_nc():
    global _NC
    if _NC is None:
        _NC = _build_nc()
    return _NC


def _prepare(inputs):
    e4 = ml_dtypes.float8_e4m3
    x = np.asarray(inputs["x"], dtype=np.float32)
    adj = np.asarray(inputs["adj"], dtype=np.float32)
    W = np.asarray(inputs["W"], dtype=np.float32)
    b = np.asarray(inputs["b"], dtype=np.float64)

    y = x @ W.T.astype(np.float32)
    y8 = y.astype(e4)
    # bias fold: nn bias + exact centering/rank-1 correction term
    c = (b + 0.5 * y.astype(np.float64).sum(axis=0)).astype(np.float32)
    cb_tiled = np.ascontiguousarray(c.reshape(NT, P).T)  # [128, 4]

    # y pre-tiled so every y DMA is flat: yp[p, k*DIM+d] = y8[k*128+p, d]
    yp = np.ascontiguousarray(
        y8.reshape(KT, P, DIM).transpose(1, 0, 2).reshape(P, KT * DIM)
    )

    B8T = (adj - np.float32(0.5)).astype(e4).T  # [K, rows] view

    in_maps = []
    for ci in range(NCORES):
        # adjp[t, p, mc, i, mm] = B8T[(2t+i)*128+p, ci*M + mc*512+mm]
        shard = np.ascontiguousarray(B8T[:, ci * M : (ci + 1) * M])
        adjp = np.ascontiguousarray(
            shard.reshape(DT, 2, P, MCH, FREE).transpose(0, 2, 3, 1, 4)
        )
        in_maps.append({"yp": yp, "adjp": adjp, "cb": cb_tiled})
    return in_maps


def _run(in_maps, **kwargs):
    return run_bass_kernel_spmd(
        _get_nc(), in_maps, core_ids=list(range(NCORES)), **kwargs
    )


def _assemble(results):
    out = np.empty((N_NODES, DIM), dtype=np.float32)
    for ci in range(NCORES):
        o5 = results[ci]["out5"].astype(np.float32)  # [MCH, NT//2, P, 2, FREE]
        outT = o5.transpose(1, 3, 2, 0, 4).reshape(DIM, M)
        out[ci * M : (ci + 1) * M, :] = outT.T
    return out


def kernel(**inputs):
    res = _run(_prepare(inputs))
    return _assemble(res.results)
